# revision 2
# baseline (speedup 1.0000x reference)
"""CameraExtrinsics Trainium2 Bass kernel.

reference:
    t = translation[i]          # (rays, 3)
    new_o = o + t
    R_all = expm(skew(rotation))  # (200, 3, 3)
    R = R_all[i]                # (rays, 3, 3)
    new_d = einsum("nij,nj->ni", R, d)
    return (new_o, new_d, R, t)

Strategy (data-parallel over rays, 8 cores):
  - Host: Rodrigues formula gives R_all exactly (O(200) work), build a
    pair-record table ptable[(i1*200+i2)] = [R(i1).flat, t(i1), R(i2).flat,
    t(i2)] (40000 x 24 f32, 3.84 MB) so one DMA-gather descriptor fetches
    records for TWO consecutive rays (96B vs 48B -> half the per-descriptor
    floor cost on the SDMA array).
  - Device per core (524288 rays): stream i/o/d, compute pair keys on DVE,
    gather 96B pair records via SWDGE indirect DMA, repack into R/t tiles
    (DVE strided copies), new_o = o + t, new_d = R @ d (15 strided DVE
    elementwise ops), stream out the four outputs with big contiguous DMAs.
  - Everything double-buffered under TileContext.
"""

import os

import numpy as np

RAYS = 4194304
N_IMAGES = 200
N_CORES = 8
RPC = RAYS // N_CORES  # 524288 rays per core
P = 128

_module_cache = {}


def _rodrigues(w64):
    """w64: (N,3) float64 -> (N,3,3) float64 rotation matrices (= expm(skew(w)))."""
    n = w64.shape[0]
    th = np.linalg.norm(w64, axis=1)  # (N,)
    wx, wy, wz = w64[:, 0], w64[:, 1], w64[:, 2]
    z = np.zeros_like(wx)
    K = np.stack(
        [
            np.stack([z, -wz, wy], -1),
            np.stack([wz, z, -wx], -1),
            np.stack([-wy, wx, z], -1),
        ],
        -2,
    )  # (N,3,3)
    K2 = K @ K
    # sin(th)/th and (1-cos th)/th^2 with series fallback near zero
    small = th < 1e-8
    th_safe = np.where(small, 1.0, th)
    a = np.where(small, 1.0 - th**2 / 6.0, np.sin(th_safe) / th_safe)
    b = np.where(small, 0.5 - th**2 / 24.0, (1.0 - np.cos(th_safe)) / th_safe**2)
    eye = np.broadcast_to(np.eye(3), (n, 3, 3))
    return eye + a[:, None, None] * K + b[:, None, None] * K2


def _build_module(rpc, w):
    """Build + compile the per-core SPMD Bass module.

    rpc: rays per core; w: rays per partition per tile (even).
    """
    from contextlib import ExitStack

    import concourse.bacc as bacc
    import concourse.bass as bass
    import concourse.mybir as mybir
    import concourse.tile as tile

    f32 = mybir.dt.float32
    i32 = mybir.dt.int32
    mult = mybir.AluOpType.mult
    add = mybir.AluOpType.add

    assert w % 2 == 0
    F = w // 2  # pairs per partition per tile
    assert rpc % (P * w) == 0
    ntiles = rpc // (P * w)

    nc = bacc.Bacc("TRN2", target_bir_lowering=False, num_devices=N_CORES)

    i_d = nc.dram_tensor("i32", [rpc], i32, kind="ExternalInput").ap()
    o_d = nc.dram_tensor("o", [rpc * 3], f32, kind="ExternalInput").ap()
    d_d = nc.dram_tensor("d", [rpc * 3], f32, kind="ExternalInput").ap()
    tab = nc.dram_tensor(
        "ptable", [N_IMAGES * N_IMAGES, 24], f32, kind="ExternalInput"
    ).ap()
    no_d = nc.dram_tensor("new_o", [rpc * 3], f32, kind="ExternalOutput").ap()
    nd_d = nc.dram_tensor("new_d", [rpc * 3], f32, kind="ExternalOutput").ap()
    R_d = nc.dram_tensor("R", [rpc * 9], f32, kind="ExternalOutput").ap()
    t_d = nc.dram_tensor("t", [rpc * 3], f32, kind="ExternalOutput").ap()

    iv = i_d.rearrange("(k p w) -> k p w", k=ntiles, p=P)
    ov = o_d.rearrange("(k p w) -> k p w", k=ntiles, p=P)
    dv = d_d.rearrange("(k p w) -> k p w", k=ntiles, p=P)
    nov = no_d.rearrange("(k p w) -> k p w", k=ntiles, p=P)
    ndv = nd_d.rearrange("(k p w) -> k p w", k=ntiles, p=P)
    Rv = R_d.rearrange("(k p w) -> k p w", k=ntiles, p=P)
    tv = t_d.rearrange("(k p w) -> k p w", k=ntiles, p=P)

    with tile.TileContext(nc) as tc, ExitStack() as ctx:
        pool = ctx.enter_context(tc.tile_pool(name="io", bufs=2))
        for k in range(ntiles):
            it = pool.tile([P, w], i32, tag="it")
            nc.sync.dma_start(it[:], iv[k])
            ot = pool.tile([P, 3 * w], f32, tag="ot")
            nc.sync.dma_start(ot[:], ov[k])
            dtile = pool.tile([P, 3 * w], f32, tag="dt")
            nc.sync.dma_start(dtile[:], dv[k])

            # pair keys: key[p, f] = i[p, 2f] * 200 + i[p, 2f+1]
            itf = pool.tile([P, w], f32, tag="itf")
            nc.vector.tensor_copy(itf[:], it[:])
            itf2 = itf[:].rearrange("p (f two) -> p f two", two=2)
            keyf = pool.tile([P, F], f32, tag="keyf")
            nc.vector.scalar_tensor_tensor(
                out=keyf[:].rearrange("p (f one) -> p f one", one=1),
                in0=itf2[:, :, 0:1],
                scalar=float(N_IMAGES),
                in1=itf2[:, :, 1:2],
                op0=mult,
                op1=add,
            )
            kt = pool.tile([P, F], i32, tag="kt")
            nc.vector.tensor_copy(kt[:], keyf[:])

            # gather pair records (24 f32 each) from the DRAM table.
            # HW vector-indirect DMA consumes ONE offset per partition per
            # instruction (verified on HW), so loop over pair slots.
            pt = pool.tile([P, 24 * F], f32, tag="pt")
            for f in range(F):
                nc.gpsimd.indirect_dma_start(
                    out=pt[:, 24 * f : 24 * (f + 1)],
                    out_offset=None,
                    in_=tab[:],
                    in_offset=bass.IndirectOffsetOnAxis(
                        ap=kt[:, f : f + 1], axis=0
                    ),
                )

            pt24 = pt[:].rearrange("p (f c) -> p f c", c=24)

            # repack R: ray-major [w, 9] per partition
            Rt = pool.tile([P, 9 * w], f32, tag="Rt")
            Rt18 = Rt[:].rearrange("p (f c) -> p f c", c=18)
            nc.vector.tensor_copy(Rt18[:, :, 0:9], pt24[:, :, 0:9])
            nc.vector.tensor_copy(Rt18[:, :, 9:18], pt24[:, :, 12:21])

            # repack t: ray-major [w, 3] per partition
            tt = pool.tile([P, 3 * w], f32, tag="tt")
            tt6 = tt[:].rearrange("p (f c) -> p f c", c=6)
            nc.vector.tensor_copy(tt6[:, :, 0:3], pt24[:, :, 9:12])
            nc.vector.tensor_copy(tt6[:, :, 3:6], pt24[:, :, 21:24])

            # new_o = o + t
            noto = pool.tile([P, 3 * w], f32, tag="no")
            nc.vector.tensor_add(noto[:], ot[:], tt[:])

            # new_d[j] = R[j] @ d[j]
            ndt = pool.tile([P, 3 * w], f32, tag="nd")
            tmp = pool.tile([P, w], f32, tag="tmp")
            R9 = Rt[:].rearrange("p (w c) -> p w c", c=9)
            d3 = dtile[:].rearrange("p (w c) -> p w c", c=3)
            n3 = ndt[:].rearrange("p (w c) -> p w c", c=3)
            t1 = tmp[:].rearrange("p (w one) -> p w one", one=1)
            for r in range(3):
                nc.vector.tensor_tensor(
                    out=n3[:, :, r : r + 1],
                    in0=R9[:, :, 3 * r : 3 * r + 1],
                    in1=d3[:, :, 0:1],
                    op=mult,
                )
                for cc in (1, 2):
                    nc.vector.tensor_tensor(
                        out=t1[:],
                        in0=R9[:, :, 3 * r + cc : 3 * r + cc + 1],
                        in1=d3[:, :, cc : cc + 1],
                        op=mult,
                    )
                    nc.vector.tensor_tensor(
                        out=n3[:, :, r : r + 1],
                        in0=n3[:, :, r : r + 1],
                        in1=t1[:],
                        op=add,
                    )

            nc.sync.dma_start(nov[k], noto[:])
            nc.sync.dma_start(ndv[k], ndt[:])
            nc.sync.dma_start(Rv[k], Rt[:])
            nc.sync.dma_start(tv[k], tt[:])

    nc.compile()
    return nc


def _get_module(rpc, w):
    key = (rpc, w)
    if key not in _module_cache:
        _module_cache[key] = _build_module(rpc, w)
    return _module_cache[key]


def _host_tables(rotation, translation):
    R_all = _rodrigues(np.asarray(rotation, np.float64)).astype(np.float32)
    rec = np.concatenate(
        [R_all.reshape(N_IMAGES, 9), np.asarray(translation, np.float32)], axis=1
    )  # (200, 12)
    left = np.broadcast_to(rec[:, None, :], (N_IMAGES, N_IMAGES, 12))
    right = np.broadcast_to(rec[None, :, :], (N_IMAGES, N_IMAGES, 12))
    ptable = np.ascontiguousarray(
        np.concatenate([left, right], axis=-1).reshape(N_IMAGES * N_IMAGES, 24)
    ).astype(np.float32)
    return ptable


def kernel(rotation, translation, i, o, d):
    from concourse.bass_utils import run_bass_kernel_spmd

    trace = bool(int(os.environ.get("CAM_KERNEL_TRACE", "0")))
    w = int(os.environ.get("CAM_KERNEL_W", "512"))

    ptable = _host_tables(rotation, translation)
    i32 = np.ascontiguousarray(np.asarray(i).astype(np.int32))
    o32 = np.ascontiguousarray(np.asarray(o, np.float32)).reshape(-1)
    d32 = np.ascontiguousarray(np.asarray(d, np.float32)).reshape(-1)

    nc = _get_module(RPC, w)

    in_maps = []
    for c in range(N_CORES):
        sl = slice(c * RPC, (c + 1) * RPC)
        sl3 = slice(c * RPC * 3, (c + 1) * RPC * 3)
        in_maps.append(
            {
                "i32": i32[sl],
                "o": o32[sl3],
                "d": d32[sl3],
                "ptable": ptable,
            }
        )

    res = run_bass_kernel_spmd(
        nc, in_maps, core_ids=list(range(N_CORES)), trace=trace
    )
    if trace and res.exec_time_ns is not None:
        print(f"HW exec time: {res.exec_time_ns} ns")

    new_o = np.concatenate([res.results[c]["new_o"] for c in range(N_CORES)]).reshape(
        RAYS, 3
    )
    new_d = np.concatenate([res.results[c]["new_d"] for c in range(N_CORES)]).reshape(
        RAYS, 3
    )
    R = np.concatenate([res.results[c]["R"] for c in range(N_CORES)]).reshape(
        RAYS, 3, 3
    )
    t = np.concatenate([res.results[c]["t"] for c in range(N_CORES)]).reshape(RAYS, 3)
    return (new_o, new_d, R, t)
